# revision 24
# baseline (speedup 1.0000x reference)
"""GAT layer on 8 TRN2 cores — V4: gather x-rows (256 B) in transpose mode,
recompute h per edge on TensorE; self-loops handled without gather.

Design:
  - dst windows (128 output nodes each, 391 global) bin-packed onto 8 cores
    by per-window tile count so the SPMD program shape matches all cores.
  - Per non-self edge, gather the 256 B bf16 x-row of its src node from a
    host-staged [N, 128] table via gpsimd dma_gather(transpose=True): output
    lands as [128 c, slots e] — directly the lhsT layout for the h matmul.
  - int16 gather indices use a mid-table base (row 32768) so negative
    indices cover rows [0, 32768) and positives [32768, 50000): one stream.
  - Per 128-edge tile: ps_he = xe_tile^T @ Wext ([128 e, 264] PSUM: 256 h,
    4 a_src-logit, 4 a_dst-logit-weight cols); a_dst of the edge's target
    is accumulated into cols 256:260 by a second (one-hot ohF) matmul;
    e = exp(leakyrelu(a_src + a_dst)); msg = h * e; one-hot ohT matmul
    segment-sums msg + denominators into PSUM.
  - The reference's appended self-loops are excluded from the edge list;
    their contribution (ex_self, ex_self*h_own) is computed from an
    SBUF-resident per-window h table (built once from xoT) and added at
    normalize time. Then divide by denominators, add bias.
  - Critical path = gather descriptor-gen on GpSimd (~9.3 ns/idx); all
    PE/Vector/Scalar work hides under it. Gathers start at t~=0.
"""
import sys
sys.path.insert(0, '/opt/trn_rl_repo')
import numpy as np
import ml_dtypes

import bass_rust as _br
import concourse.bacc as bacc
import concourse.mybir as mybir
import concourse.tile as tile
from concourse import bass_utils

BF16 = ml_dtypes.bfloat16
FP8 = ml_dtypes.float8_e4m3

C_IN = 128
C_OUT_TOT = 256   # HEADS * OUT_CH
HEADS = 4
HC = 64
NEG_SLOPE = 0.2
MID = 32768       # gather index base row
G = 2             # windows per gather call


def host_prep(x, edge_index, W, att_src, att_dst, bias, n_cores=8):
    """Shard + schedule. Returns (cfg, in_maps, core_wins)."""
    N = x.shape[0]
    src = np.asarray(edge_index[0], np.int64).astype(np.int32)
    dst = np.asarray(edge_index[1], np.int64).astype(np.int32)

    NWG = (N + 127) // 128                     # global windows
    NW = (NWG + n_cores - 1) // n_cores        # window slots per core

    order = np.argsort(dst, kind='stable')
    src_s, dst_s = src[order], dst[order]
    w_lo = np.searchsorted(dst_s, np.arange(NWG) * 128, 'left')
    w_hi = np.searchsorted(dst_s, (np.arange(NWG) + 1) * 128, 'left')
    w_cnt = w_hi - w_lo
    # windows whose edges all have src < MID need >=1 pad slot so the
    # runtime trailing-negative strip can't eat real indices
    w_maxsrc = np.full(NWG, -1, np.int64)
    np.maximum.at(w_maxsrc, dst_s // 128, src_s)
    w_need_pad = ((w_cnt % 128 == 0) & (w_cnt > 0) & (w_maxsrc < MID)).astype(np.int64)
    w_tiles = np.maximum((w_cnt + w_need_pad + 127) // 128, 1)

    # bin-pack windows onto cores: biggest-first to least-loaded core
    core_wins = [[] for _ in range(n_cores)]
    core_load = np.zeros(n_cores, np.int64)
    for wid in np.argsort(-w_tiles, kind='stable'):
        c = int(np.argmin(core_load))
        core_wins[c].append(int(wid))
        core_load[c] += w_tiles[wid]

    # per-slot tile count: max across cores (SPMD shape)
    T_slot = np.ones(NW, np.int64)
    for c in range(n_cores):
        real = sorted(core_wins[c], key=lambda w: -w_tiles[w])
        core_wins[c] = real + [-1] * (NW - len(real))
        for k, w in enumerate(core_wins[c]):
            if w >= 0:
                T_slot[k] = max(T_slot[k], w_tiles[w])

    NCALL = (NW + G - 1) // G
    call_slots = []
    for k in range(NCALL):
        call_slots.append(int(T_slot[k * G:(k + 1) * G].sum()) * 128)
    TOT = sum(call_slots)

    cfg = dict(N=N, n_cores=n_cores, NW=NW, NCALL=NCALL,
               T_slot=[int(t) for t in T_slot], call_slots=call_slots,
               TOT=TOT)

    xT16 = np.ascontiguousarray(np.asarray(x, np.float32)).astype(BF16)  # [N,128]
    W_b = np.asarray(W, np.float32).astype(BF16)           # [128, 256]
    WT_b = np.ascontiguousarray(np.asarray(W).T).astype(BF16)  # [256, 128]
    att_flatT = np.zeros((C_OUT_TOT, 2 * HEADS), np.float32)
    for h in range(HEADS):
        att_flatT[h * HC:(h + 1) * HC, h] = np.asarray(att_src)[h]
        att_flatT[h * HC:(h + 1) * HC, HEADS + h] = np.asarray(att_dst)[h]
    att_flatT_b = att_flatT.astype(BF16)                   # [256, 8]
    bias_bc = np.broadcast_to(np.asarray(bias, np.float32), (128, C_OUT_TOT)).copy()
    ident8 = np.eye(128, dtype=np.float32).astype(FP8)     # [128, 128]
    iota8 = np.broadcast_to(np.arange(128, dtype=np.int8), (128, 128)).copy()
    xTfull = xT16.T                                        # [128, N] view

    in_maps = []
    for c in range(n_cores):
        idx = np.zeros(TOT, np.int16)
        dstl8 = np.full((128, TOT // 128), -1, np.int8)
        ohF = np.zeros((128, TOT), FP8)
        xoT = np.zeros((128, NW * 128), BF16)
        col = 0
        for k, wid in enumerate(core_wins[c]):
            ts = int(T_slot[k]) * 128
            if wid >= 0:
                nn = min(128, N - wid * 128)
                xoT[:, k * 128:k * 128 + nn] = xTfull[:, wid * 128:wid * 128 + nn]
                sw = src_s[w_lo[wid]:w_hi[wid]].copy()
                dw = dst_s[w_lo[wid]:w_hi[wid]].copy() - wid * 128
                n = len(sw)
                assert n <= ts
                idx[col:col + n] = (sw - MID).astype(np.int16)
                # trailing-negative guard: last element of window block
                if n == ts and n > 0 and sw[n - 1] < MID:
                    pos = np.nonzero(sw >= MID)[0]
                    assert len(pos), "window with all-src<MID and full tiles"
                    p = pos[-1]
                    idx[col + p], idx[col + n - 1] = idx[col + n - 1], idx[col + p]
                    sw[p], sw[n - 1] = sw[n - 1], sw[p]
                    dw[p], dw[n - 1] = dw[n - 1], dw[p]
                e_pos = np.arange(n)
                lanes = e_pos % 128
                tiles = e_pos // 128
                dstl8[lanes, col // 128 + tiles] = dw.astype(np.int8)
                ohF[dw, col + tiles * 128 + lanes] = 1.0
            col += ts
        assert col == TOT
        wrapped = idx.reshape(TOT // 16, 16).T             # [16, TOT/16]
        idx16 = np.tile(wrapped, (8, 1)).copy()            # [128, TOT/16]
        in_maps.append({
            "xtab": xT16, "xoT": xoT,
            "Wb": W_b, "WTb": WT_b, "attT": att_flatT_b, "bias_bc": bias_bc,
            "ident8": ident8, "iota8": iota8, "dstl8": dstl8,
            "idx16": idx16, "ohF": ohF,
        })
    return cfg, in_maps, core_wins


def build_program(cfg):
    N, NW, NCALL, TOT = (cfg[k] for k in ("N", "NW", "NCALL", "TOT"))
    T_slot, call_slots = cfg["T_slot"], cfg["call_slots"]
    n_cores = cfg["n_cores"]
    MAXSLOTS = max(call_slots)
    dt = mybir.dt

    nc = bacc.Bacc("TRN2", target_bir_lowering=False, debug=False,
                   num_devices=n_cores)
    t_xtab = nc.dram_tensor("xtab", (N, C_IN), dt.bfloat16, kind="ExternalInput")
    t_xoT = nc.dram_tensor("xoT", (C_IN, NW * 128), dt.bfloat16, kind="ExternalInput")
    t_Wb = nc.dram_tensor("Wb", (C_IN, C_OUT_TOT), dt.bfloat16, kind="ExternalInput")
    t_WTb = nc.dram_tensor("WTb", (C_OUT_TOT, C_IN), dt.bfloat16, kind="ExternalInput")
    t_attT = nc.dram_tensor("attT", (C_OUT_TOT, 2 * HEADS), dt.bfloat16, kind="ExternalInput")
    t_bias = nc.dram_tensor("bias_bc", (128, C_OUT_TOT), dt.float32, kind="ExternalInput")
    t_idx = nc.dram_tensor("idx16", (128, TOT // 16), dt.int16, kind="ExternalInput")
    t_ohF = nc.dram_tensor("ohF", (128, TOT), dt.float8e4, kind="ExternalInput")
    t_dstl = nc.dram_tensor("dstl8", (128, TOT // 128), dt.int8, kind="ExternalInput")
    t_iota = nc.dram_tensor("iota8", (128, 128), dt.int8, kind="ExternalInput")
    t_id8 = nc.dram_tensor("ident8", (128, 128), dt.float8e4, kind="ExternalInput")
    t_out = nc.dram_tensor("out", (NW * 128, C_OUT_TOT), dt.float32, kind="ExternalOutput")

    ap_mid = t_xtab.ap()[MID:N, :]
    EXT = C_OUT_TOT + 2 * HEADS      # 264
    call_off = [0]
    for cs in call_slots:
        call_off.append(call_off[-1] + cs)

    with tile.TileContext(nc) as tc:
        with tc.tile_pool(name="const", bufs=1) as cpool, \
             tc.tile_pool(name="xep", bufs=3) as xep, \
             tc.tile_pool(name="idxp", bufs=3) as idxp, \
             tc.tile_pool(name="ohp", bufs=2) as ohp:

            def issue_call_loads(k):
                cs = call_slots[k]
                c0 = call_off[k]
                idxc = idxp.tile([128, MAXSLOTS // 16], dt.int16, tag="idxc",
                                 name="idxc")
                nc.sync.dma_start(out=idxc[:, 0:cs // 16],
                                  in_=t_idx.ap()[:, c0 // 16:(c0 + cs) // 16])
                xe = xep.tile([128, 1, MAXSLOTS], dt.bfloat16, tag="xe", name="xe")
                nc.gpsimd.dma_gather(
                    out_ap=xe[:, :, 0:cs], in_ap=ap_mid,
                    idxs_ap=idxc[:, 0:cs // 16],
                    num_idxs=cs, num_idxs_reg=cs, elem_size=C_IN,
                    transpose=True, single_packet=False,
                )
                ohF_b = ohp.tile([128, MAXSLOTS], dt.float8e4, tag="ohF",
                                 name="ohF_b")
                for q in range(4):
                    q0 = q * cs // 4 // 128 * 128
                    q1 = cs if q == 3 else (q + 1) * cs // 4 // 128 * 128
                    nc.sync.dma_start(out=ohF_b[:, q0:q1],
                                      in_=t_ohF.ap()[:, c0 + q0:c0 + q1])
                return xe, ohF_b

            cur = issue_call_loads(0)     # gathers start at t~=0
            dstl_sb = cpool.tile([128, TOT // 128], dt.int8)
            nc.sync.dma_start(out=dstl_sb, in_=t_dstl.ap())
            iota_sb = cpool.tile([128, 128], dt.int8)
            nc.sync.dma_start(out=iota_sb, in_=t_iota.ap())

            Wext_sb = cpool.tile([C_IN, EXT], dt.bfloat16)
            nc.sync.dma_start(out=Wext_sb[:, 0:C_OUT_TOT], in_=t_Wb.ap())
            bias_sb = cpool.tile([128, C_OUT_TOT], dt.float32)
            nc.sync.dma_start(out=bias_sb, in_=t_bias.ap())
            ident_sb = cpool.tile([128, 128], dt.float8e4)
            nc.sync.dma_start(out=ident_sb, in_=t_id8.ap())
            xoT_sb = cpool.tile([C_IN, NW * 128], dt.bfloat16)
            for q in range(4):
                q0, q1 = q * NW * 32, (q + 1) * NW * 32
                nc.sync.dma_start(out=xoT_sb[:, q0:q1], in_=t_xoT.ap()[:, q0:q1])

            # w_att = W @ att_flatT -> Wext cols 256:264
            with tc.tile_pool(name="watt_ps", bufs=1, space="PSUM") as wpp, \
                 tc.tile_pool(name="watt_sb", bufs=1) as wsp:
                ps_watt = wpp.tile([C_IN, 2 * HEADS], dt.float32)
                wt0 = wsp.tile([128, C_IN], dt.bfloat16)
                wt1 = wsp.tile([128, C_IN], dt.bfloat16)
                at0 = wsp.tile([128, 2 * HEADS], dt.bfloat16)
                at1 = wsp.tile([128, 2 * HEADS], dt.bfloat16)
                nc.sync.dma_start(out=wt0, in_=t_WTb.ap()[0:128, :])
                nc.sync.dma_start(out=wt1, in_=t_WTb.ap()[128:256, :])
                nc.sync.dma_start(out=at0, in_=t_attT.ap()[0:128, :])
                nc.sync.dma_start(out=at1, in_=t_attT.ap()[128:256, :])
                nc.tensor.matmul(out=ps_watt, lhsT=wt0, rhs=at0, start=True, stop=False)
                nc.tensor.matmul(out=ps_watt, lhsT=wt1, rhs=at1, start=False, stop=True)
                nc.vector.tensor_copy(out=Wext_sb[:, C_OUT_TOT:EXT], in_=ps_watt)

            # ---------- main: gather + per-tile pipeline ----------
            with tc.tile_pool(name="msgp", bufs=3) as msgp, \
                 tc.tile_pool(name="ohtp", bufs=3) as ohtp, \
                 tc.tile_pool(name="sp", bufs=4) as sp, \
                 tc.tile_pool(name="hps", bufs=4, space="PSUM") as hps, \
                 tc.tile_pool(name="wps", bufs=2, space="PSUM") as wps:
                for k in range(NCALL):
                    xe, ohF_b = cur
                    if k + 1 < NCALL:
                        cur = issue_call_loads(k + 1)

                    ccol = 0      # call-local slot offset
                    for w in range(k * G, min((k + 1) * G, NW)):
                        T = T_slot[w]
                        # own-node h/logit row for this window (a_dst source
                        # + self-loop message), computed inline
                        ps_o = wps.tile([128, EXT], dt.float32, tag="ps_o")
                        nc.tensor.matmul(out=ps_o,
                                         lhsT=xoT_sb[:, w * 128:(w + 1) * 128],
                                         rhs=Wext_sb, start=True, stop=True)
                        hs = sp.tile([128, EXT], dt.bfloat16, tag="hs")
                        nc.scalar.copy(out=hs, in_=ps_o)
                        gt0 = (call_off[k] + ccol) // 128
                        oht = ohtp.tile([128, T, 128], dt.float8e4, tag="oht",
                                        name=f"oht{T}")
                        for t0c in range(0, T, 4):
                            ch = min(4, T - t0c)
                            nc.vector.tensor_tensor(
                                out=oht[:, t0c:t0c + ch, :],
                                in0=dstl_sb[:, gt0 + t0c:gt0 + t0c + ch]
                                    .unsqueeze(2).broadcast_to([128, ch, 128]),
                                in1=iota_sb.unsqueeze(1).broadcast_to(
                                    [128, ch, 128]),
                                op=mybir.AluOpType.is_equal)
                        msg = msgp.tile([128, T, EXT], dt.bfloat16, tag="msg",
                                        name=f"msg{T}")
                        for t in range(T):
                            ps_he = hps.tile([128, EXT], dt.float32, tag="ps_he")
                            nc.tensor.matmul(
                                out=ps_he,
                                lhsT=xe[:, 0, ccol + t * 128:ccol + (t + 1) * 128],
                                rhs=Wext_sb, start=True, stop=False,
                                skip_group_check=True)
                            # accumulate a_dst[dst(e)] into the a_src logit cols
                            nc.tensor.matmul(
                                out=ps_he[:, 256:260],
                                lhsT=ohF_b[:, ccol + t * 128:ccol + (t + 1) * 128],
                                rhs=hs[:, 260:264],
                                start=False, stop=True, skip_group_check=True)
                            e2 = sp.tile([128, HEADS], dt.float32, tag="e2")
                            nc.scalar.activation(out=e2, in_=ps_he[:, 256:260],
                                                 func=mybir.ActivationFunctionType.Prelu,
                                                 alpha=NEG_SLOPE)
                            nc.scalar.activation(out=msg[:, t, 256:260], in_=e2,
                                                 func=mybir.ActivationFunctionType.Exp)
                            exb = msg[:, t, 256:260].unsqueeze(2).broadcast_to(
                                [128, HEADS, HC])
                            nc.vector.tensor_tensor(
                                out=msg[:, t, 0:C_OUT_TOT].rearrange(
                                    "p (h c) -> p h c", h=HEADS),
                                in0=ps_he[:, 0:C_OUT_TOT].rearrange(
                                    "p (h c) -> p h c", h=HEADS),
                                in1=exb, op=mybir.AluOpType.mult)

                        # self-loop message tile: cols 0:256 = h_own*ex_self,
                        # 256:260 = ex_self (rides the same aggregation matmul)
                        zs = sp.tile([128, HEADS], dt.float32, tag="zs")
                        nc.vector.tensor_tensor(out=zs, in0=hs[:, 256:260],
                                                in1=hs[:, 260:264],
                                                op=mybir.AluOpType.add)
                        zp = sp.tile([128, HEADS], dt.float32, tag="zp")
                        nc.scalar.activation(out=zp, in_=zs,
                                             func=mybir.ActivationFunctionType.Prelu,
                                             alpha=NEG_SLOPE)
                        msgs = sp.tile([128, 260], dt.bfloat16, tag="msgs")
                        nc.scalar.activation(out=msgs[:, 256:260], in_=zp,
                                             func=mybir.ActivationFunctionType.Exp)
                        exsb = msgs[:, 256:260].unsqueeze(2).broadcast_to(
                            [128, HEADS, HC])
                        nc.vector.tensor_tensor(
                            out=msgs[:, 0:C_OUT_TOT].rearrange(
                                "p (h c) -> p h c", h=HEADS),
                            in0=hs[:, 0:C_OUT_TOT].rearrange(
                                "p (h c) -> p h c", h=HEADS),
                            in1=exsb, op=mybir.AluOpType.mult)

                        ps_win = wps.tile([128, 260], dt.float32, tag="ps_win")
                        for t in range(T):
                            nc.tensor.matmul(
                                out=ps_win,
                                lhsT=oht[:, t, :],
                                rhs=msg[:, t, 0:260],
                                start=(t == 0), stop=False)
                        nc.tensor.matmul(out=ps_win, lhsT=ident_sb, rhs=msgs,
                                         start=False, stop=True)

                        rcp = sp.tile([128, HEADS], dt.float32, tag="rcp")
                        nc.vector.reciprocal(out=rcp, in_=ps_win[:, 256:260])
                        osb = sp.tile([128, C_OUT_TOT], dt.float32, tag="osb")
                        for h in range(HEADS):
                            nc.vector.tensor_scalar(
                                out=osb[:, h * HC:(h + 1) * HC],
                                in0=ps_win[:, h * HC:(h + 1) * HC],
                                scalar1=rcp[:, h:h + 1], scalar2=None,
                                op0=mybir.AluOpType.mult)
                        nc.vector.tensor_tensor(out=osb, in0=osb, in1=bias_sb,
                                                op=mybir.AluOpType.add)
                        nc.sync.dma_start(out=t_out.ap()[w * 128:(w + 1) * 128, :],
                                          in_=osb)
                        ccol += T * 128

    nc.finalize()
    return nc


def register_ntff_hook():
    import types
    import antenv
    if getattr(antenv, 'axon_hooks', None) is not None:
        return
    mod = types.ModuleType('antenv.axon_hooks')
    _hook = [None]
    mod.set_axon_ntff_profile_hook = lambda h: _hook.__setitem__(0, h)
    mod.get_axon_ntff_profile_hook = lambda: _hook[0]
    sys.modules['antenv.axon_hooks'] = mod
    antenv.axon_hooks = mod
    try:
        from trn_agent_boot.trn_boot import _ntff_profile_via_ctypes
        mod.set_axon_ntff_profile_hook(
            _ntff_profile_via_ctypes('/opt/axon/libaxon_pjrt.so'))
    except Exception:
        pass


def run(x, edge_index, W, att_src, att_dst, bias, n_cores=8, trace=False):
    cfg, in_maps, core_wins = host_prep(x, edge_index, W, att_src, att_dst,
                                        bias, n_cores)
    nc = build_program(cfg)
    if trace:
        register_ntff_hook()
    r = bass_utils.run_bass_kernel_spmd(nc, in_maps,
                                        core_ids=list(range(n_cores)),
                                        trace=trace)
    N = cfg["N"]
    out = np.empty((N, C_OUT_TOT), np.float32)
    for c in range(n_cores):
        oc = r.results[c]["out"]
        for k, wid in enumerate(core_wins[c]):
            if wid < 0:
                continue
            nn = min(128, N - wid * 128)
            out[wid * 128:wid * 128 + nn, :] = oc[k * 128:k * 128 + nn, :]
    return out, r


# ----------------------------------------------------------------------------
# Self-contained harness entry point: full inputs in, full output out.
# ----------------------------------------------------------------------------
import os as _os


def kernel(x, edge_index, W, att_src, att_dst, bias):
    x = np.asarray(x, np.float32)
    edge_index = np.asarray(edge_index)
    W = np.asarray(W, np.float32)
    att_src = np.asarray(att_src, np.float32)
    att_dst = np.asarray(att_dst, np.float32)
    bias = np.asarray(bias, np.float32)
    trace = _os.environ.get("GAT_TRACE", "0") == "1"
    out, r = run(x, edge_index, W, att_src, att_dst, bias, n_cores=8, trace=trace)
    if trace and r.exec_time_ns is not None:
        print(f"HW exec time: {r.exec_time_ns} ns")
    return np.ascontiguousarray(out.astype(np.float32))
